# revision 8
# baseline (speedup 1.0000x reference)
"""Trainium2 Bass kernel for nn_DisBlock (Swin-style window-attention transformer block).

Strategy: data-parallel over the B=128 window/batch dim across 8 NeuronCores
(16 batches per core, processed as 8 pairs of 2 batches = 512 tokens). Host
work is limited to input staging: slicing, weight transposition/quantization,
and the rel-pos bias gather rp_table[rel_index] (pure indexing of two inputs).

Datapath (per core):
  - activations x stay f32 in [token_p, C_f]; LN stats on DVE (bn_stats),
    rstd = exp(-0.5*ln(var+eps)) on Act so LN+softmax+LN2 share one
    activation table (natural_log_exp); only gelu switches tables.
  - normalized activations are written as fp8e4 packed in uint16 tiles,
    transposed to contraction-major layout by the DMA xbar transpose
    (2-byte granularity = channel pairs), which lands exactly in the
    [p, 2, f] operand layout of fp8 DoubleRow matmuls (K=256/instruction).
  - weights are quantized host-side to fp8e4 * 64 (g1/g2 folded in); the
    1/64 descale is folded into psum evictions (Act scale / DVE tensor_scalar).
  - per-token bias/noise terms (b1, noise*ns via w_qkv, b_proj) are seeded
    into the matmul PSums with tiny K<=2/K=1 matmuls instead of DVE adds;
    the rel-pos softmax bias is seeded into the score PSum with an
    identity-matmul copy, so softmax is exp(psum) directly (Act, fp8 out).
  - softmax runs unnormalized in transposed layout S^T[m,n]; row sums come
    from a ones-column PV matmul; 1/sum is applied after PV per head.
  - MLP: fc1 -> gelu(fp8) into one [128,16,512] tile, fc2 accumulates per
    token tile in a single PSum (keeps PSUM pressure low).
"""

import os

import numpy as np

_STAGES = int(os.environ.get("K_STAGES", "9"))  # debug bisection knob

B, N, C, H, W = 128, 256, 512, 8, 16
D = C // H
HID = 4 * C
SCALE = float(D) ** -0.5
EPS = 1e-5
NCORES = 8
BL = B // NCORES          # batches per core
NPAIR = BL // 2           # batch pairs per core
NT = 4                    # token tiles (128) per pair
KC = C // 128             # contraction tiles over C
KH = HID // 128           # contraction tiles over HID
WS = 64.0                 # fp8 weight scale

_CACHE = {}


def _build_nc():
    import concourse.bacc as bacc
    import concourse.mybir as mybir
    import concourse.tile as tile

    f32 = mybir.dt.float32
    bf16 = mybir.dt.bfloat16
    fp8 = mybir.dt.float8e4
    u16 = mybir.dt.uint16
    AF = mybir.ActivationFunctionType
    OP = mybir.AluOpType
    PM = mybir.MatmulPerfMode

    nc = bacc.Bacc("TRN2", target_bir_lowering=False, debug=False)

    # Steer the greedy act-table picker to the ln+exp union set so LN rstd
    # (ln,exp) and softmax exp share one table: empty the single-function
    # sets that would otherwise match first. Set ids are positional, so
    # only set CONTENTS may change, never the order.
    from concourse.hw_specs import get_activation_tables
    tabs = get_activation_tables(nc.m.arch)
    if "natural_log_exp_and_others" in tabs:
        for shadowed in ("natural_log", "exp_and_others", "exp_and_friends"):
            if shadowed in tabs:
                tabs[shadowed] = set()

    # ---- DRAM I/O ----
    xin = nc.dram_tensor("xin", [BL, N, C], f32, kind="ExternalInput")
    d_nzT = nc.dram_tensor("nzT", [NPAIR, 2, 2 * N], bf16, kind="ExternalInput")
    d_wqkv8 = nc.dram_tensor("wqkv8", [128, KC, 3 * C], fp8, kind="ExternalInput")
    d_wproj8 = nc.dram_tensor("wproj8", [128, KC, C], fp8, kind="ExternalInput")
    d_w18 = nc.dram_tensor("w18", [128, KC, HID], fp8, kind="ExternalInput")
    d_w28 = nc.dram_tensor("w28", [128, KH, C], fp8, kind="ExternalInput")
    d_seedqkv = nc.dram_tensor("seedqkv", [2, 3 * C], bf16, kind="ExternalInput")
    d_biasT = nc.dram_tensor("biasT", [128, 2, H, N], bf16, kind="ExternalInput")
    d_b1mt = nc.dram_tensor("b1mt", [128, KH], f32, kind="ExternalInput")
    d_b2mb = nc.dram_tensor("b2mb", [128, C], f32, kind="ExternalInput")
    d_bprojW = nc.dram_tensor("bprojW", [1, C], bf16, kind="ExternalInput")
    d_id = nc.dram_tensor("identb", [128, 128], bf16, kind="ExternalInput")
    yout = nc.dram_tensor("yout", [BL, N, C], f32, kind="ExternalOutput")

    with tile.TileContext(nc) as tc:
        with (
            tc.tile_pool(name="const", bufs=1) as cpool,
            tc.tile_pool(name="xt", bufs=2) as xpool,
            tc.tile_pool(name="h8", bufs=2) as hpool,
            tc.tile_pool(name="o8", bufs=2) as opool,
            tc.tile_pool(name="ht", bufs=2) as htpool,
            tc.tile_pool(name="qkvT", bufs=4) as qkpool,
            tc.tile_pool(name="vaug", bufs=2) as vpool,
            tc.tile_pool(name="pt", bufs=18) as ptpool,
            tc.tile_pool(name="gt", bufs=2) as gpool,
            tc.tile_pool(name="y", bufs=2) as ypool,
            tc.tile_pool(name="nz", bufs=2) as nzpool,
            tc.tile_pool(name="small", bufs=4) as spool,
            tc.tile_pool(name="ps_mm", bufs=2, space="PSUM") as pmm,
            tc.tile_pool(name="ps_s", bufs=2, space="PSUM") as pss,
            tc.tile_pool(name="ps_pv", bufs=2, space="PSUM") as ppv,
            tc.tile_pool(name="ps_p2", bufs=2, space="PSUM") as pp2,
        ):
            # ---- resident constants ----
            wqkv8 = cpool.tile([128, KC, 3 * C], fp8, tag="wqkv8")
            wproj8 = cpool.tile([128, KC, C], fp8, tag="wproj8")
            w18 = cpool.tile([128, KC, HID], fp8, tag="w18")
            w28 = cpool.tile([128, KH, C], fp8, tag="w28")
            seedq = cpool.tile([2, 3 * C], bf16, tag="seedq")
            biasT = cpool.tile([128, 2, H, N], bf16, tag="biasT")
            b1mt = cpool.tile([128, KH], f32, tag="b1mt")
            b2mb = cpool.tile([128, C], f32, tag="b2mb")
            bprojW = cpool.tile([1, C], bf16, tag="bprojW")
            identb = cpool.tile([128, 128], bf16, tag="identb")
            onescol = cpool.tile([1, 128], bf16, tag="onescol")
            epsb = cpool.tile([128, 1], f32, tag="epsb")
            nc.gpsimd.memset(epsb[:], EPS)
            nc.gpsimd.memset(onescol[:], 1.0)
            for t, d in [
                (identb, d_id), (wqkv8, d_wqkv8), (biasT, d_biasT),
                (seedq, d_seedqkv), (wproj8, d_wproj8), (bprojW, d_bprojW),
                (w18, d_w18), (b1mt, d_b1mt), (w28, d_w28), (b2mb, d_b2mb),
            ]:
                nc.sync.dma_start(t[:], d[:])

            # vaug double buffer with persistent ones columns
            vaugs = []
            for i in range(2):
                v = vpool.tile([128, NT, 66 * H], fp8, tag="vaug")
                ones_cols = v[:].rearrange("p t (h x) -> p t h x", x=66)[:, :, :, 64:65]
                nc.gpsimd.memset(ones_cols, 1.0)
                vaugs.append(v)

            def layernorm_block(dst, src):
                # dst[:, tt, :] = (src[:, tt, :] - m) * rstd
                stats = spool.tile([128, NT, 2], f32, tag="stats")
                for tt in range(NT):
                    st6 = spool.tile([128, 6], f32, tag="st6")
                    nc.vector.bn_stats(st6[:], src[:, tt, :])
                    nc.vector.bn_aggr(stats[:, tt, :], st6[:])
                lnv = spool.tile([128, NT], f32, tag="lnv")
                nc.scalar.activation(lnv[:], stats[:, :, 1], AF.Ln, bias=epsb[:])
                rstd = spool.tile([128, NT], f32, tag="rstd")
                nc.scalar.activation(rstd[:], lnv[:], AF.Exp, scale=-0.5)
                for tt in range(NT):
                    nc.vector.tensor_scalar(
                        dst[:, tt, :], src[:, tt, :],
                        stats[:, tt, 0:1], rstd[:, tt:tt + 1],
                        op0=OP.subtract, op1=OP.mult,
                    )

            def transpose_cast(tb, t8, srcb):
                # srcb [128, NT, C] bf16 -> tb [128, KC, 2N] bf16 (dma xbar)
                # -> t8 [128, KC, 2N] fp8 (DVE cast per token chunk)
                for tt in range(NT):
                    nc.sync.dma_start_transpose(
                        tb[:, :, 128 * tt:128 * tt + 128], srcb[:, tt, :]
                    )
                for half in range(2):
                    nc.vector.tensor_copy(
                        t8[:, :, 256 * half:256 * half + 256],
                        tb[:, :, 256 * half:256 * half + 256],
                    )

            for p in range(NPAIR):
                b0 = 2 * p
                vaug = vaugs[p % 2]
                # ---- load x, nzT ----
                xt = xpool.tile([128, NT, C], f32, tag="xt")
                for j in range(2):
                    nc.scalar.dma_start(
                        xt[:, 2 * j:2 * j + 2, :],
                        xin[b0 + j].rearrange("(t p) c -> p t c", p=128),
                    )
                nzt = nzpool.tile([2, 2 * N], bf16, tag="nzt")
                nc.scalar.dma_start(nzt[:], d_nzT[p])

                # ---- LN1 -> h (bf16) ----
                hb = hpool.tile([128, NT, C], bf16, tag="hb")
                layernorm_block(hb[:], xt)

                # ---- transpose h -> hTb -> hT8 ----
                hTb = htpool.tile([128, KC, 2 * N], bf16, tag="hTb")
                hT8 = htpool.tile([128, KC, 2 * N], fp8, tag="hT8")
                transpose_cast(hTb, hT8, hb)

                # ---- v ----
                for mt in range(NT):
                    ps = pmm.tile([128, 512], f32, tag="mm")
                    nc.tensor.matmul(
                        ps[:], nzt[:, 128 * mt:128 * mt + 128],
                        seedq[:, 2 * C:3 * C], start=True, stop=False,
                    )
                    for g in range(2):
                        nc.tensor.matmul(
                            ps[:],
                            hT8[:, 2 * g:2 * g + 2, 128 * mt:128 * mt + 128],
                            wqkv8[:, 2 * g:2 * g + 2, 2 * C:3 * C],
                            start=False, stop=(g == 1), perf_mode=PM.DoubleRow,
                        )
                    nc.vector.tensor_scalar(
                        vaug[:, mt, :].rearrange("p (h x) -> p h x", x=66)[:, :, 0:64],
                        ps[:].rearrange("p (h x) -> p h x", x=64),
                        1.0 / WS, None, op0=OP.mult,
                    )

                # ---- q, k per head group ----
                qkvTs = []
                for hg in range(2):
                    qkvT = qkpool.tile([128, 4, 2 * N], bf16, tag="qkvT")
                    for i, et in enumerate([2 * hg, 2 * hg + 1, 4 + 2 * hg, 5 + 2 * hg]):
                        ps = pmm.tile([128, 512], f32, tag="mm")
                        nc.tensor.matmul(
                            ps[:], seedq[:, 128 * et:128 * et + 128], nzt[:],
                            start=True, stop=False,
                        )
                        for g in range(2):
                            nc.tensor.matmul(
                                ps[:],
                                wqkv8[:, 2 * g:2 * g + 2, 128 * et:128 * et + 128],
                                hT8[:, 2 * g:2 * g + 2, :],
                                start=False, stop=(g == 1), perf_mode=PM.DoubleRow,
                            )
                        if i < 2:  # q: fold attention scale; Act evict
                            nc.scalar.activation(
                                qkvT[:, i, :], ps[:], AF.Copy, scale=SCALE / WS
                            )
                        else:      # k: DVE evict
                            nc.vector.tensor_scalar(
                                qkvT[:, i, :], ps[:], 1.0 / WS, None, op0=OP.mult
                            )
                    qkvTs.append(qkvT)

                if _STAGES < 2:
                    for tt in range(NT):
                        y = ypool.tile([128, C], f32, tag="y")
                        nc.vector.tensor_copy(y[:], xt[:, tt, :])
                        bi, nt = b0 + tt // 2, tt % 2
                        nc.sync.dma_start(yout[bi, 128 * nt:128 * nt + 128, :], y[:])
                    continue

                # ---- attention ----
                ob = opool.tile([128, NT, C], bf16, tag="ob")
                for bb in range(2):
                    pts = []
                    for hh in range(H):
                        hg, j = hh // 4, hh % 4
                        qkvT = qkvTs[hg]
                        qi, ki, poff = j // 2, 2 + j // 2, 64 * (j % 2)
                        ps_s = pss.tile([128, 2, N], f32, tag="s")
                        for mi in range(2):
                            nc.tensor.matmul(
                                ps_s[:, mi, :], identb[:], biasT[:, mi, hh, :],
                                start=True, stop=False,
                            )
                            mt = 2 * bb + mi
                            nc.tensor.matmul(
                                ps_s[:, mi, :],
                                qkvT[poff:poff + 64, ki, 128 * mt:128 * mt + 128],
                                qkvT[poff:poff + 64, qi, N * bb:N * bb + N],
                                start=False, stop=True,
                            )
                        pt = ptpool.tile([128, 2, N], fp8, tag="pt")
                        nc.scalar.activation(pt[:], ps_s[:], AF.Exp)
                        pts.append(pt)
                    for nt in range(2):
                        po = ppv.tile([128, H, 64], f32, tag="pv")
                        po2 = pp2.tile([128, H], f32, tag="pv2")
                        for hh in range(H):
                            lhsT = pts[hh][:, :, 128 * nt:128 * nt + 128]
                            nc.tensor.matmul(
                                po[:, hh, :], lhsT,
                                vaug[:, 2 * bb:2 * bb + 2, 66 * hh:66 * hh + 64],
                                start=True, stop=True, perf_mode=PM.DoubleRow,
                            )
                            nc.tensor.matmul(
                                po2[:, hh:hh + 1], lhsT,
                                vaug[:, 2 * bb:2 * bb + 2, 64:65],
                                start=True, stop=True, perf_mode=PM.DoubleRow,
                            )
                        inv = spool.tile([128, H], f32, tag="inv")
                        nc.vector.reciprocal(inv[:], po2[:])
                        tt = 2 * bb + nt
                        for hh in range(H):
                            if hh % 2 == 0:
                                nc.vector.tensor_scalar(
                                    ob[:, tt, 64 * hh:64 * hh + 64],
                                    po[:, hh, :], inv[:, hh:hh + 1], None,
                                    op0=OP.mult,
                                )
                            else:
                                nc.scalar.activation(
                                    ob[:, tt, 64 * hh:64 * hh + 64],
                                    po[:, hh, :], AF.Copy,
                                    scale=inv[:, hh:hh + 1],
                                )

                if _STAGES < 3:
                    for tt in range(NT):
                        y = ypool.tile([128, C], f32, tag="y")
                        nc.vector.tensor_copy(y[:], ob[:, tt, :])
                        bi, nt = b0 + tt // 2, tt % 2
                        nc.sync.dma_start(yout[bi, 128 * nt:128 * nt + 128, :], y[:])
                    continue

                # ---- transpose o; proj; residual into xt ----
                oTb = htpool.tile([128, KC, 2 * N], bf16, tag="oTb")
                oT8 = htpool.tile([128, KC, 2 * N], fp8, tag="oT8")
                transpose_cast(oTb, oT8, ob)
                for tt in range(NT):
                    ps = pmm.tile([128, 512], f32, tag="mm")
                    nc.tensor.matmul(
                        ps[:], onescol[:], bprojW[:], start=True, stop=False
                    )
                    for g in range(2):
                        nc.tensor.matmul(
                            ps[:],
                            oT8[:, 2 * g:2 * g + 2, 128 * tt:128 * tt + 128],
                            wproj8[:, 2 * g:2 * g + 2, :],
                            start=False, stop=(g == 1), perf_mode=PM.DoubleRow,
                        )
                    nc.vector.scalar_tensor_tensor(
                        xt[:, tt, :], ps[:], 1.0 / WS, xt[:, tt, :],
                        op0=OP.mult, op1=OP.add,
                    )

                if _STAGES < 4:
                    for tt in range(NT):
                        y = ypool.tile([128, C], f32, tag="y")
                        nc.vector.tensor_copy(y[:], xt[:, tt, :])
                        bi, nt = b0 + tt // 2, tt % 2
                        nc.sync.dma_start(yout[bi, 128 * nt:128 * nt + 128, :], y[:])
                    continue

                # ---- LN2 -> h2; transpose ----
                h2b = hpool.tile([128, NT, C], bf16, tag="hb")
                layernorm_block(h2b[:], xt)
                h2Tb = htpool.tile([128, KC, 2 * N], bf16, tag="hTb")
                h2T8 = htpool.tile([128, KC, 2 * N], fp8, tag="hT8")
                transpose_cast(h2Tb, h2T8, h2b)

                # ---- MLP ----
                gt8 = gpool.tile([128, KH, 2 * N], fp8, tag="gt8")
                for t_ in range(KH):
                    ps = pmm.tile([128, 512], f32, tag="mm")
                    for g in range(2):
                        nc.tensor.matmul(
                            ps[:],
                            w18[:, 2 * g:2 * g + 2, 128 * t_:128 * t_ + 128],
                            h2T8[:, 2 * g:2 * g + 2, :],
                            start=(g == 0), stop=(g == 1), perf_mode=PM.DoubleRow,
                        )
                    nc.scalar.activation(
                        gt8[:, t_, :], ps[:], AF.Gelu,
                        bias=b1mt[:, t_:t_ + 1], scale=1.0 / WS,
                    )
                for tt in range(NT):
                    psy = pmm.tile([128, 512], f32, tag="mm")
                    for r in range(KH // 2):
                        nc.tensor.matmul(
                            psy[:],
                            gt8[:, 2 * r:2 * r + 2, 128 * tt:128 * tt + 128],
                            w28[:, 2 * r:2 * r + 2, :],
                            start=(r == 0), stop=(r == KH // 2 - 1),
                            perf_mode=PM.DoubleRow,
                        )
                    y = ypool.tile([128, C], f32, tag="y")
                    nc.vector.scalar_tensor_tensor(
                        y[:], psy[:], 1.0 / WS, b2mb[:], op0=OP.mult, op1=OP.add
                    )
                    nc.gpsimd.tensor_add(y[:], y[:], xt[:, tt, :])
                    bi, nt = b0 + tt // 2, tt % 2
                    nc.sync.dma_start(yout[bi, 128 * nt:128 * nt + 128, :], y[:])

    nc.compile()
    return nc


def _host_prep(x, noise, ns, g1, b1, w_qkv, w_proj, b_proj, rp_table, g2, b2,
               w1, b1m, w2, b2m, rel_index):
    import ml_dtypes
    f = np.float32
    e4 = ml_dtypes.float8_e4m3
    bf = ml_dtypes.bfloat16

    w_qkv = np.asarray(w_qkv, f)
    w_proj = np.asarray(w_proj, f)
    w1 = np.asarray(w1, f)
    w2 = np.asarray(w2, f)
    g1 = np.asarray(g1, f)
    g2 = np.asarray(g2, f)
    b1 = np.asarray(b1, f)
    b2 = np.asarray(b2, f)

    # rel-pos bias gather -> [p, mi, h, n] with m = mi*128+p, bias[n, m, h]
    bias = np.asarray(rp_table, f)[np.asarray(rel_index).reshape(-1)]
    bias = bias.reshape(N, N, H)
    biasT = np.ascontiguousarray(
        bias.transpose(1, 0, 2).reshape(2, 128, N, H).transpose(1, 0, 3, 2)
    ).astype(bf)

    def tiled_T(wf, kt):
        # wf [out, cin] -> [128, kt, out]: [p, k, :] = wf[:, 128k+p]
        wt = np.ascontiguousarray(wf.T)
        return np.ascontiguousarray(
            wt.reshape(kt, 128, wt.shape[1]).transpose(1, 0, 2)
        )

    wq_f = w_qkv * g1[None, :]          # fold g1
    w1_f = w1 * g2[None, :]             # fold g2

    wqkv8 = (WS * tiled_T(wq_f, KC)).astype(e4)
    wproj8 = (WS * tiled_T(w_proj, KC)).astype(e4)
    w18 = (WS * tiled_T(w1_f, KC)).astype(e4)
    w28 = (WS * tiled_T(w2, KH)).astype(e4)

    wb = w_qkv @ b1                      # [3C]
    wsum = w_qkv.sum(axis=1)             # [3C]
    seedqkv = np.ascontiguousarray(
        (WS * np.stack([wb, wsum])).astype(bf)
    )

    b1m_eff = np.asarray(b1m, f) + w1 @ b2
    b1mt = np.ascontiguousarray(b1m_eff.reshape(KH, 128).T)
    b2mb = np.ascontiguousarray(np.broadcast_to(
        np.asarray(b2m, f).reshape(1, -1), (128, C)))
    bprojW = (WS * np.asarray(b_proj, f).reshape(1, C)).astype(bf)

    shared = {
        "wqkv8": wqkv8, "wproj8": wproj8, "w18": w18, "w28": w28,
        "seedqkv": seedqkv, "biasT": biasT, "b1mt": b1mt, "b2mb": b2mb,
        "bprojW": bprojW, "identb": np.eye(128, dtype=f).astype(bf),
    }
    x = np.asarray(x, f)
    nz = np.asarray(noise, f).reshape(B, N) * np.float32(ns)
    in_maps = []
    for c in range(NCORES):
        m = dict(shared)
        m["xin"] = np.ascontiguousarray(x[c * BL:(c + 1) * BL])
        nzc = nz[c * BL:(c + 1) * BL].reshape(NPAIR, 2 * N)
        nzT = np.ones((NPAIR, 2, 2 * N), f)
        nzT[:, 1, :] = nzc
        m["nzT"] = nzT.astype(bf)
        in_maps.append(m)
    return in_maps


def kernel(**inputs):
    from concourse.bass_utils import run_bass_kernel_spmd

    if "nc" not in _CACHE:
        _CACHE["nc"] = _build_nc()
    nc = _CACHE["nc"]
    import time as _time

    in_maps = _host_prep(**inputs)
    _t0 = _time.time()
    res = run_bass_kernel_spmd(nc, in_maps, core_ids=list(range(NCORES)))
    _CACHE["last_run_s"] = _time.time() - _t0
    out = np.concatenate([res.results[c]["yout"] for c in range(NCORES)], axis=0)
    return out.astype(np.float32)


# revision 12
# speedup vs baseline: 1.0723x; 1.0723x over previous
"""Trainium2 Bass kernel for nn_DisBlock (Swin-style window-attention transformer block).

Strategy: data-parallel over the B=128 window/batch dim across 8 NeuronCores
(16 batches per core, processed as 8 pairs of 2 batches = 512 tokens). Host
work is limited to input staging: slicing, weight transposition/quantization,
and the rel-pos bias gather rp_table[rel_index] (pure indexing of two inputs).

Datapath (per core):
  - activations x stay f32 in [token_p, C_f]; LN stats on DVE (bn_stats),
    rstd = exp(-0.5*ln(var+eps)) on Act so LN+softmax+LN2 share one
    activation table (natural_log_exp); only gelu switches tables.
  - normalized activations are written as fp8e4 packed in uint16 tiles,
    transposed to contraction-major layout by the DMA xbar transpose
    (2-byte granularity = channel pairs), which lands exactly in the
    [p, 2, f] operand layout of fp8 DoubleRow matmuls (K=256/instruction).
  - weights are quantized host-side to fp8e4 * 64 (g1/g2 folded in); the
    1/64 descale is folded into psum evictions (Act scale / DVE tensor_scalar).
  - per-token bias/noise terms (b1, noise*ns via w_qkv, b_proj) are seeded
    into the matmul PSums with tiny K<=2/K=1 matmuls instead of DVE adds;
    the rel-pos softmax bias is seeded into the score PSum with an
    identity-matmul copy, so softmax is exp(psum) directly (Act, fp8 out).
  - softmax runs unnormalized in transposed layout S^T[m,n]; row sums come
    from a ones-column PV matmul; 1/sum is applied after PV per head.
  - MLP: fc1 -> gelu(fp8) into one [128,16,512] tile, fc2 accumulates per
    token tile in a single PSum (keeps PSUM pressure low).
"""

import os

import numpy as np

_STAGES = int(os.environ.get("K_STAGES", "9"))  # debug bisection knob

B, N, C, H, W = 128, 256, 512, 8, 16
D = C // H
HID = 4 * C
SCALE = float(D) ** -0.5
EPS = 1e-5
NCORES = 8
BL = B // NCORES          # batches per core
NPAIR = BL // 2           # batch pairs per core
NT = 4                    # token tiles (128) per pair
KC = C // 128             # contraction tiles over C
KH = HID // 128           # contraction tiles over HID
WS = 64.0                 # fp8 weight scale

_CACHE = {}


def _build_nc():
    import concourse.bacc as bacc
    import concourse.mybir as mybir
    import concourse.tile as tile

    f32 = mybir.dt.float32
    bf16 = mybir.dt.bfloat16
    fp8 = mybir.dt.float8e4
    u16 = mybir.dt.uint16
    AF = mybir.ActivationFunctionType
    OP = mybir.AluOpType
    PM = mybir.MatmulPerfMode

    nc = bacc.Bacc("TRN2", target_bir_lowering=False, debug=False)

    # Steer the greedy act-table picker to the ln+exp union set so LN rstd
    # (ln,exp) and softmax exp share one table: empty the single-function
    # sets that would otherwise match first. Set ids are positional, so
    # only set CONTENTS may change, never the order.
    from concourse.hw_specs import get_activation_tables
    tabs = get_activation_tables(nc.m.arch)
    if "natural_log_exp_and_others" in tabs:
        for shadowed in ("natural_log", "exp_and_others", "exp_and_friends"):
            if shadowed in tabs:
                tabs[shadowed] = set()

    # ---- DRAM I/O ----
    xin = nc.dram_tensor("xin", [BL, N, C], f32, kind="ExternalInput")
    d_nzT = nc.dram_tensor("nzT", [NPAIR, 2, 2 * N], bf16, kind="ExternalInput")
    d_wqkv8 = nc.dram_tensor("wqkv8", [128, KC, 3 * C], fp8, kind="ExternalInput")
    d_wproj8 = nc.dram_tensor("wproj8", [128, KC, C], fp8, kind="ExternalInput")
    d_w18 = nc.dram_tensor("w18", [128, KC, HID], fp8, kind="ExternalInput")
    d_w28 = nc.dram_tensor("w28", [128, KH, C], fp8, kind="ExternalInput")
    d_seedqkv = nc.dram_tensor("seedqkv", [2, 3 * C], bf16, kind="ExternalInput")
    d_biasT = nc.dram_tensor("biasT", [128, 2, H, N], bf16, kind="ExternalInput")
    d_b1mt = nc.dram_tensor("b1mt", [128, KH], f32, kind="ExternalInput")
    d_b2mb = nc.dram_tensor("b2mb", [128, C], f32, kind="ExternalInput")
    d_bprojW = nc.dram_tensor("bprojW", [1, C], bf16, kind="ExternalInput")
    d_id = nc.dram_tensor("identb", [128, 128], bf16, kind="ExternalInput")
    yout = nc.dram_tensor("yout", [BL, N, C], f32, kind="ExternalOutput")

    with tile.TileContext(nc) as tc:
        with (
            tc.tile_pool(name="const", bufs=1) as cpool,
            tc.tile_pool(name="xt", bufs=3) as xpool,
            tc.tile_pool(name="h8", bufs=2) as hpool,
            tc.tile_pool(name="o8", bufs=2) as opool,
            tc.tile_pool(name="ht", bufs=2) as htpool,
            tc.tile_pool(name="qkvT", bufs=4) as qkpool,
            tc.tile_pool(name="vaug", bufs=2) as vpool,
            tc.tile_pool(name="pt", bufs=18) as ptpool,
            tc.tile_pool(name="gt", bufs=2) as gpool,
            tc.tile_pool(name="y", bufs=2) as ypool,
            tc.tile_pool(name="nz", bufs=3) as nzpool,
            tc.tile_pool(name="small", bufs=4) as spool,
            tc.tile_pool(name="ps_mm", bufs=3, space="PSUM") as pmm,
            tc.tile_pool(name="ps_s", bufs=2, space="PSUM") as pss,
            tc.tile_pool(name="ps_pv", bufs=2, space="PSUM") as ppv,
            tc.tile_pool(name="ps_p2", bufs=1, space="PSUM") as pp2,
        ):
            # ---- resident constants ----
            wqkv8 = cpool.tile([128, KC, 3 * C], fp8, tag="wqkv8")
            wproj8 = cpool.tile([128, KC, C], fp8, tag="wproj8")
            w18 = cpool.tile([128, KC, HID], fp8, tag="w18")
            w28 = cpool.tile([128, KH, C], fp8, tag="w28")
            seedq = cpool.tile([2, 3 * C], bf16, tag="seedq")
            biasT = cpool.tile([128, 2, H, N], bf16, tag="biasT")
            b1mt = cpool.tile([128, KH], f32, tag="b1mt")
            b2mb = cpool.tile([128, C], f32, tag="b2mb")
            bprojW = cpool.tile([1, C], bf16, tag="bprojW")
            identb = cpool.tile([128, 128], bf16, tag="identb")
            onescol = cpool.tile([1, 128], bf16, tag="onescol")
            epsb = cpool.tile([128, 1], f32, tag="epsb")
            nc.gpsimd.memset(epsb[:], EPS)
            nc.gpsimd.memset(onescol[:], 1.0)
            for t, d in [
                (identb, d_id), (wqkv8, d_wqkv8), (biasT, d_biasT),
                (seedq, d_seedqkv), (wproj8, d_wproj8), (bprojW, d_bprojW),
                (w18, d_w18), (b1mt, d_b1mt), (w28, d_w28), (b2mb, d_b2mb),
            ]:
                nc.sync.dma_start(t[:], d[:])

            # vaug double buffer with persistent ones columns
            vaugs = []
            for i in range(2):
                v = vpool.tile([128, NT, 66 * H], fp8, tag="vaug")
                ones_cols = v[:].rearrange("p t (h x) -> p t h x", x=66)[:, :, :, 64:65]
                nc.gpsimd.memset(ones_cols, 1.0)
                vaugs.append(v)

            def layernorm_block(dst, src):
                # dst[:, tt, :] = (src[:, tt, :] - m) * rstd
                stats = spool.tile([128, NT, 2], f32, tag="stats")
                for tt in range(NT):
                    st6 = spool.tile([128, 6], f32, tag="st6")
                    nc.vector.bn_stats(st6[:], src[:, tt, :])
                    nc.vector.bn_aggr(stats[:, tt, :], st6[:])
                lnv = spool.tile([128, NT], f32, tag="lnv")
                nc.scalar.activation(lnv[:], stats[:, :, 1], AF.Ln, bias=epsb[:])
                rstd = spool.tile([128, NT], f32, tag="rstd")
                nc.scalar.activation(rstd[:], lnv[:], AF.Exp, scale=-0.5)
                for tt in range(NT):
                    nc.vector.tensor_scalar(
                        dst[:, tt, :], src[:, tt, :],
                        stats[:, tt, 0:1], rstd[:, tt:tt + 1],
                        op0=OP.subtract, op1=OP.mult,
                    )

            def transpose_cast(tb, t8, srcb):
                # srcb [128, NT, C] bf16 -> tb [128, KC, 2N] bf16 (dma xbar)
                # -> t8 [128, KC, 2N] fp8 (DVE cast per token chunk)
                for tt in range(NT):
                    nc.sync.dma_start_transpose(
                        tb[:, :, 128 * tt:128 * tt + 128], srcb[:, tt, :]
                    )
                for half in range(2):
                    nc.vector.tensor_copy(
                        t8[:, :, 256 * half:256 * half + 256],
                        tb[:, :, 256 * half:256 * half + 256],
                    )

            # ---- software-pipelined emission over pairs ----
            # iteration i emits: beta_attn(i), alpha_ln(i+1), beta_proj(i),
            # alpha_qkv(i+1), beta_ln2(i), gamma(i) so pair i+1's LN/transpose
            # chain and pair i's Act-paced phases overlap in each engine's
            # in-order stream. Act-table regions stay [ln_exp ... | gelu].
            S = [dict() for _ in range(NPAIR)]

            def load(p):
                s = S[p]
                s["xt"] = xpool.tile([128, NT, C], f32, name=f"xt{p}", tag="xt")
                for j in range(2):
                    nc.gpsimd.dma_start(
                        s["xt"][:, 2 * j:2 * j + 2, :],
                        xin[2 * p + j].rearrange("(t p) c -> p t c", p=128),
                    )
                s["nzt"] = nzpool.tile([2, 2 * N], bf16, name=f"nzt{p}", tag="nzt")
                nc.gpsimd.dma_start(s["nzt"][:], d_nzT[p])

            def alpha_ln(p):
                s = S[p]
                hb = hpool.tile([128, NT, C], bf16, tag="hb")
                layernorm_block(hb[:], s["xt"])
                s["hTb"] = htpool.tile([128, KC, 2 * N], bf16, name=f"hTb{p}", tag="hTb")
                for tt in range(NT):
                    nc.sync.dma_start_transpose(
                        s["hTb"][:, :, 128 * tt:128 * tt + 128], hb[:, tt, :]
                    )

            def alpha_qkv(p):
                s = S[p]
                nzt = s["nzt"]
                vaug = vaugs[p % 2]
                s["vaug"] = vaug
                hT8 = htpool.tile([128, KC, 2 * N], fp8, tag="hT8")
                for half in range(2):
                    nc.vector.tensor_copy(
                        hT8[:, :, 256 * half:256 * half + 256],
                        s["hTb"][:, :, 256 * half:256 * half + 256],
                    )
                for mt in range(NT):
                    ps = pmm.tile([128, 512], f32, tag="mm")
                    nc.tensor.matmul(
                        ps[:], nzt[:, 128 * mt:128 * mt + 128],
                        seedq[:, 2 * C:3 * C], start=True, stop=False,
                    )
                    for g in range(2):
                        nc.tensor.matmul(
                            ps[:],
                            hT8[:, 2 * g:2 * g + 2, 128 * mt:128 * mt + 128],
                            wqkv8[:, 2 * g:2 * g + 2, 2 * C:3 * C],
                            start=False, stop=(g == 1), perf_mode=PM.DoubleRow,
                        )
                    nc.vector.tensor_scalar(
                        vaug[:, mt, :].rearrange("p (h x) -> p h x", x=66)[:, :, 0:64],
                        ps[:].rearrange("p (h x) -> p h x", x=64),
                        1.0 / WS, None, op0=OP.mult,
                    )
                qkvTs = []
                for hg in range(2):
                    qkvT = qkpool.tile([128, 4, 2 * N], bf16, tag="qkvT")
                    for i, et in enumerate([2 * hg, 2 * hg + 1, 4 + 2 * hg, 5 + 2 * hg]):
                        ps = pmm.tile([128, 512], f32, tag="mm")
                        nc.tensor.matmul(
                            ps[:], seedq[:, 128 * et:128 * et + 128], nzt[:],
                            start=True, stop=False,
                        )
                        for g in range(2):
                            nc.tensor.matmul(
                                ps[:],
                                wqkv8[:, 2 * g:2 * g + 2, 128 * et:128 * et + 128],
                                hT8[:, 2 * g:2 * g + 2, :],
                                start=False, stop=(g == 1), perf_mode=PM.DoubleRow,
                            )
                        if i < 2:  # q: fold attention scale; Act evict
                            nc.scalar.activation(
                                qkvT[:, i, :], ps[:], AF.Copy, scale=SCALE / WS
                            )
                        else:      # k: DVE evict
                            nc.vector.tensor_scalar(
                                qkvT[:, i, :], ps[:], 1.0 / WS, None, op0=OP.mult
                            )
                    qkvTs.append(qkvT)
                s["qkvTs"] = qkvTs

            def beta_attn(p):
                s = S[p]
                vaug = s["vaug"]
                ob = opool.tile([128, NT, C], bf16, name=f"ob{p}", tag="ob")
                s["ob"] = ob
                po2t = pp2.tile([128, 4, H], f32, tag="pv2")
                for bb in range(2):
                    pts = []
                    for hh in range(H):
                        hg, j = hh // 4, hh % 4
                        qkvT = s["qkvTs"][hg]
                        qi, ki, poff = j // 2, 2 + j // 2, 64 * (j % 2)
                        ps_s = pss.tile([128, 2, N], f32, tag="s")
                        for mi in range(2):
                            nc.tensor.matmul(
                                ps_s[:, mi, :], identb[:], biasT[:, mi, hh, :],
                                start=True, stop=False,
                            )
                            mt = 2 * bb + mi
                            nc.tensor.matmul(
                                ps_s[:, mi, :],
                                qkvT[poff:poff + 64, ki, 128 * mt:128 * mt + 128],
                                qkvT[poff:poff + 64, qi, N * bb:N * bb + N],
                                start=False, stop=True,
                            )
                        pt = ptpool.tile([128, 2, N], fp8, tag="pt")
                        nc.scalar.activation(pt[:], ps_s[:], AF.Exp)
                        pts.append(pt)
                    for nt in range(2):
                        po = ppv.tile([128, H, 64], f32, tag="pv")
                        for hh in range(H):
                            lhsT = pts[hh][:, :, 128 * nt:128 * nt + 128]
                            nc.tensor.matmul(
                                po[:, hh, :], lhsT,
                                vaug[:, 2 * bb:2 * bb + 2, 66 * hh:66 * hh + 64],
                                start=True, stop=True, perf_mode=PM.DoubleRow,
                            )
                            nc.tensor.matmul(
                                po2t[:, 2 * bb + nt, hh:hh + 1], lhsT,
                                vaug[:, 2 * bb:2 * bb + 2, 64:65],
                                start=True, stop=True, perf_mode=PM.DoubleRow,
                            )
                        inv = spool.tile([128, H], f32, tag="inv")
                        nc.vector.reciprocal(inv[:], po2t[:, 2 * bb + nt, :])
                        tt = 2 * bb + nt
                        for hh in range(H):
                            if hh % 2 == 0:
                                nc.vector.tensor_scalar(
                                    ob[:, tt, 64 * hh:64 * hh + 64],
                                    po[:, hh, :], inv[:, hh:hh + 1], None,
                                    op0=OP.mult,
                                )
                            else:
                                nc.scalar.activation(
                                    ob[:, tt, 64 * hh:64 * hh + 64],
                                    po[:, hh, :], AF.Copy,
                                    scale=inv[:, hh:hh + 1],
                                )

            def beta_proj(p):
                s = S[p]
                oTb = htpool.tile([128, KC, 2 * N], bf16, tag="oTb")
                oT8 = htpool.tile([128, KC, 2 * N], fp8, tag="oT8")
                transpose_cast(oTb, oT8, s["ob"])
                for tt in range(NT):
                    ps = pmm.tile([128, 512], f32, tag="mm")
                    nc.tensor.matmul(
                        ps[:], onescol[:], bprojW[:], start=True, stop=False
                    )
                    for g in range(2):
                        nc.tensor.matmul(
                            ps[:],
                            oT8[:, 2 * g:2 * g + 2, 128 * tt:128 * tt + 128],
                            wproj8[:, 2 * g:2 * g + 2, :],
                            start=False, stop=(g == 1), perf_mode=PM.DoubleRow,
                        )
                    nc.vector.scalar_tensor_tensor(
                        s["xt"][:, tt, :], ps[:], 1.0 / WS, s["xt"][:, tt, :],
                        op0=OP.mult, op1=OP.add,
                    )

            def beta_ln2(p):
                s = S[p]
                h2b = hpool.tile([128, NT, C], bf16, tag="hb")
                layernorm_block(h2b[:], s["xt"])
                h2Tb = htpool.tile([128, KC, 2 * N], bf16, tag="h2Tb")
                h2T8 = htpool.tile([128, KC, 2 * N], fp8, tag="h2T8")
                transpose_cast(h2Tb, h2T8, h2b)
                s["h2T8"] = h2T8

            def gamma(p):
                s = S[p]
                h2T8 = s["h2T8"]
                gt8 = gpool.tile([128, KH, 2 * N], fp8, tag="gt8")
                for t_ in range(KH):
                    ps = pmm.tile([128, 512], f32, tag="mm")
                    for g in range(2):
                        nc.tensor.matmul(
                            ps[:],
                            w18[:, 2 * g:2 * g + 2, 128 * t_:128 * t_ + 128],
                            h2T8[:, 2 * g:2 * g + 2, :],
                            start=(g == 0), stop=(g == 1), perf_mode=PM.DoubleRow,
                        )
                    nc.scalar.activation(
                        gt8[:, t_, :], ps[:], AF.Gelu,
                        bias=b1mt[:, t_:t_ + 1], scale=1.0 / WS,
                    )
                for tt in range(NT):
                    psy = pmm.tile([128, 512], f32, tag="mm")
                    for r in range(KH // 2):
                        nc.tensor.matmul(
                            psy[:],
                            gt8[:, 2 * r:2 * r + 2, 128 * tt:128 * tt + 128],
                            w28[:, 2 * r:2 * r + 2, :],
                            start=(r == 0), stop=(r == KH // 2 - 1),
                            perf_mode=PM.DoubleRow,
                        )
                    y = ypool.tile([128, C], f32, tag="y")
                    nc.vector.scalar_tensor_tensor(
                        y[:], psy[:], 1.0 / WS, b2mb[:], op0=OP.mult, op1=OP.add
                    )
                    nc.gpsimd.tensor_add(y[:], y[:], s["xt"][:, tt, :])
                    bi, nt = 2 * p + tt // 2, tt % 2
                    nc.sync.dma_start(yout[bi, 128 * nt:128 * nt + 128, :], y[:])

            load(0)
            load(1)
            alpha_ln(0)
            alpha_qkv(0)
            for i in range(NPAIR):
                if i + 2 < NPAIR:
                    load(i + 2)
                beta_attn(i)
                if i + 1 < NPAIR:
                    alpha_ln(i + 1)
                beta_proj(i)
                if i + 1 < NPAIR:
                    alpha_qkv(i + 1)
                beta_ln2(i)
                gamma(i)

    nc.compile()
    return nc


def _host_prep(x, noise, ns, g1, b1, w_qkv, w_proj, b_proj, rp_table, g2, b2,
               w1, b1m, w2, b2m, rel_index):
    import ml_dtypes
    f = np.float32
    e4 = ml_dtypes.float8_e4m3
    bf = ml_dtypes.bfloat16

    w_qkv = np.asarray(w_qkv, f)
    w_proj = np.asarray(w_proj, f)
    w1 = np.asarray(w1, f)
    w2 = np.asarray(w2, f)
    g1 = np.asarray(g1, f)
    g2 = np.asarray(g2, f)
    b1 = np.asarray(b1, f)
    b2 = np.asarray(b2, f)

    # rel-pos bias gather -> [p, mi, h, n] with m = mi*128+p, bias[n, m, h]
    bias = np.asarray(rp_table, f)[np.asarray(rel_index).reshape(-1)]
    bias = bias.reshape(N, N, H)
    biasT = np.ascontiguousarray(
        bias.transpose(1, 0, 2).reshape(2, 128, N, H).transpose(1, 0, 3, 2)
    ).astype(bf)

    def tiled_T(wf, kt):
        # wf [out, cin] -> [128, kt, out]: [p, k, :] = wf[:, 128k+p]
        wt = np.ascontiguousarray(wf.T)
        return np.ascontiguousarray(
            wt.reshape(kt, 128, wt.shape[1]).transpose(1, 0, 2)
        )

    wq_f = w_qkv * g1[None, :]          # fold g1
    w1_f = w1 * g2[None, :]             # fold g2

    wqkv8 = (WS * tiled_T(wq_f, KC)).astype(e4)
    wproj8 = (WS * tiled_T(w_proj, KC)).astype(e4)
    w18 = (WS * tiled_T(w1_f, KC)).astype(e4)
    w28 = (WS * tiled_T(w2, KH)).astype(e4)

    wb = w_qkv @ b1                      # [3C]
    wsum = w_qkv.sum(axis=1)             # [3C]
    seedqkv = np.ascontiguousarray(
        (WS * np.stack([wb, wsum])).astype(bf)
    )

    b1m_eff = np.asarray(b1m, f) + w1 @ b2
    b1mt = np.ascontiguousarray(b1m_eff.reshape(KH, 128).T)
    b2mb = np.ascontiguousarray(np.broadcast_to(
        np.asarray(b2m, f).reshape(1, -1), (128, C)))
    bprojW = (WS * np.asarray(b_proj, f).reshape(1, C)).astype(bf)

    shared = {
        "wqkv8": wqkv8, "wproj8": wproj8, "w18": w18, "w28": w28,
        "seedqkv": seedqkv, "biasT": biasT, "b1mt": b1mt, "b2mb": b2mb,
        "bprojW": bprojW, "identb": np.eye(128, dtype=f).astype(bf),
    }
    x = np.asarray(x, f)
    nz = np.asarray(noise, f).reshape(B, N) * np.float32(ns)
    in_maps = []
    for c in range(NCORES):
        m = dict(shared)
        m["xin"] = np.ascontiguousarray(x[c * BL:(c + 1) * BL])
        nzc = nz[c * BL:(c + 1) * BL].reshape(NPAIR, 2 * N)
        nzT = np.ones((NPAIR, 2, 2 * N), f)
        nzT[:, 1, :] = nzc
        m["nzT"] = nzT.astype(bf)
        in_maps.append(m)
    return in_maps


def kernel(**inputs):
    from concourse.bass_utils import run_bass_kernel_spmd

    if "nc" not in _CACHE:
        _CACHE["nc"] = _build_nc()
    nc = _CACHE["nc"]
    import time as _time

    in_maps = _host_prep(**inputs)
    _t0 = _time.time()
    res = run_bass_kernel_spmd(nc, in_maps, core_ids=list(range(NCORES)))
    _CACHE["last_run_s"] = _time.time() - _t0
    out = np.concatenate([res.results[c]["yout"] for c in range(NCORES)], axis=0)
    return out.astype(np.float32)


# revision 13
# speedup vs baseline: 1.2315x; 1.1485x over previous
"""Trainium2 Bass kernel for nn_DisBlock (Swin-style window-attention transformer block).

Strategy: data-parallel over the B=128 window/batch dim across 8 NeuronCores
(16 batches per core, processed as 8 pairs of 2 batches = 512 tokens). Host
work is limited to input staging: slicing, weight transposition/quantization,
and the rel-pos bias gather rp_table[rel_index] (pure indexing of two inputs).

Datapath (per core):
  - activations x stay f32 in [token_p, C_f]; LN stats on DVE (bn_stats),
    rstd = exp(-0.5*ln(var+eps)) on Act so LN+softmax+LN2 share one
    activation table (natural_log_exp); only gelu switches tables.
  - normalized activations are written as fp8e4 packed in uint16 tiles,
    transposed to contraction-major layout by the DMA xbar transpose
    (2-byte granularity = channel pairs), which lands exactly in the
    [p, 2, f] operand layout of fp8 DoubleRow matmuls (K=256/instruction).
  - weights are quantized host-side to fp8e4 * 64 (g1/g2 folded in); the
    1/64 descale is folded into psum evictions (Act scale / DVE tensor_scalar).
  - per-token bias/noise terms (b1, noise*ns via w_qkv, b_proj) are seeded
    into the matmul PSums with tiny K<=2/K=1 matmuls instead of DVE adds;
    the rel-pos softmax bias is seeded into the score PSum with an
    identity-matmul copy, so softmax is exp(psum) directly (Act, fp8 out).
  - softmax runs unnormalized in transposed layout S^T[m,n]; row sums come
    from a ones-column PV matmul; 1/sum is applied after PV per head.
  - MLP: fc1 -> gelu(fp8) into one [128,16,512] tile, fc2 accumulates per
    token tile in a single PSum (keeps PSUM pressure low).
"""

import os

import numpy as np

_STAGES = int(os.environ.get("K_STAGES", "9"))  # debug bisection knob

B, N, C, H, W = 128, 256, 512, 8, 16
D = C // H
HID = 4 * C
SCALE = float(D) ** -0.5
EPS = 1e-5
NCORES = 8
BL = B // NCORES          # batches per core
NPAIR = BL // 2           # batch pairs per core
NT = 4                    # token tiles (128) per pair
KC = C // 128             # contraction tiles over C
KH = HID // 128           # contraction tiles over HID
WS = 64.0                 # fp8 weight scale

_CACHE = {}


def _build_nc():
    import concourse.bacc as bacc
    import concourse.mybir as mybir
    import concourse.tile as tile

    f32 = mybir.dt.float32
    bf16 = mybir.dt.bfloat16
    fp8 = mybir.dt.float8e4
    u16 = mybir.dt.uint16
    AF = mybir.ActivationFunctionType
    OP = mybir.AluOpType
    PM = mybir.MatmulPerfMode

    nc = bacc.Bacc("TRN2", target_bir_lowering=False, debug=False)

    # Steer the greedy act-table picker to the ln+exp union set so LN rstd
    # (ln,exp) and softmax exp share one table: empty the single-function
    # sets that would otherwise match first. Set ids are positional, so
    # only set CONTENTS may change, never the order.
    from concourse.hw_specs import get_activation_tables
    tabs = get_activation_tables(nc.m.arch)
    if "natural_log_exp_and_others" in tabs:
        for shadowed in ("natural_log", "exp_and_others", "exp_and_friends"):
            if shadowed in tabs:
                tabs[shadowed] = set()

    # ---- DRAM I/O ----
    xin = nc.dram_tensor("xin", [BL, N, C], f32, kind="ExternalInput")
    d_nzT = nc.dram_tensor("nzT", [NPAIR, 2, 2 * N], bf16, kind="ExternalInput")
    d_wqkv8 = nc.dram_tensor("wqkv8", [128, KC, 3 * C], fp8, kind="ExternalInput")
    d_wproj8 = nc.dram_tensor("wproj8", [128, KC, C], fp8, kind="ExternalInput")
    d_w18 = nc.dram_tensor("w18", [128, KC, HID], fp8, kind="ExternalInput")
    d_w28 = nc.dram_tensor("w28", [128, KH, C], fp8, kind="ExternalInput")
    d_seedqkv = nc.dram_tensor("seedqkv", [2, 3 * C], bf16, kind="ExternalInput")
    d_biasT = nc.dram_tensor("biasT", [128, 2, H, N], bf16, kind="ExternalInput")
    d_b1mt = nc.dram_tensor("b1mt", [128, KH], f32, kind="ExternalInput")
    d_b2mb = nc.dram_tensor("b2mb", [128, C], f32, kind="ExternalInput")
    d_bprojW = nc.dram_tensor("bprojW", [1, C], bf16, kind="ExternalInput")
    d_id = nc.dram_tensor("identb", [128, 128], bf16, kind="ExternalInput")
    yout = nc.dram_tensor("yout", [BL, N, C], f32, kind="ExternalOutput")

    with tile.TileContext(nc) as tc:
        with (
            tc.tile_pool(name="const", bufs=1) as cpool,
            tc.tile_pool(name="xt", bufs=3) as xpool,
            tc.tile_pool(name="h8", bufs=2) as hpool,
            tc.tile_pool(name="o8", bufs=2) as opool,
            tc.tile_pool(name="ht", bufs=2) as htpool,
            tc.tile_pool(name="qkvT", bufs=4) as qkpool,
            tc.tile_pool(name="vaug", bufs=2) as vpool,
            tc.tile_pool(name="pt", bufs=18) as ptpool,
            tc.tile_pool(name="gt", bufs=2) as gpool,
            tc.tile_pool(name="y", bufs=2) as ypool,
            tc.tile_pool(name="nz", bufs=3) as nzpool,
            tc.tile_pool(name="small", bufs=4) as spool,
            tc.tile_pool(name="ps_mm", bufs=3, space="PSUM") as pmm,
            tc.tile_pool(name="ps_s", bufs=2, space="PSUM") as pss,
            tc.tile_pool(name="ps_pv", bufs=2, space="PSUM") as ppv,
            tc.tile_pool(name="ps_p2", bufs=1, space="PSUM") as pp2,
        ):
            # ---- resident constants ----
            wqkv8 = cpool.tile([128, KC, 3 * C], fp8, tag="wqkv8")
            wproj8 = cpool.tile([128, KC, C], fp8, tag="wproj8")
            w18 = cpool.tile([128, KC, HID], fp8, tag="w18")
            w28 = cpool.tile([128, KH, C], fp8, tag="w28")
            seedq = cpool.tile([2, 3 * C], bf16, tag="seedq")
            biasT = cpool.tile([128, 2, H, N], bf16, tag="biasT")
            b1mt = cpool.tile([128, KH], f32, tag="b1mt")
            b2mb = cpool.tile([128, C], f32, tag="b2mb")
            bprojW = cpool.tile([1, C], bf16, tag="bprojW")
            identb = cpool.tile([128, 128], bf16, tag="identb")
            onescol = cpool.tile([1, 128], bf16, tag="onescol")
            epsb = cpool.tile([128, 1], f32, tag="epsb")
            nc.gpsimd.memset(epsb[:], EPS)
            nc.gpsimd.memset(onescol[:], 1.0)
            for t, d in [
                (identb, d_id), (wqkv8, d_wqkv8), (biasT, d_biasT),
                (seedq, d_seedqkv), (wproj8, d_wproj8), (bprojW, d_bprojW),
                (w18, d_w18), (b1mt, d_b1mt), (w28, d_w28), (b2mb, d_b2mb),
            ]:
                nc.sync.dma_start(t[:], d[:])

            # vaug double buffer with persistent ones columns
            vaugs = []
            for i in range(2):
                v = vpool.tile([128, NT, 66 * H], fp8, tag="vaug")
                ones_cols = v[:].rearrange("p t (h x) -> p t h x", x=66)[:, :, :, 64:65]
                nc.gpsimd.memset(ones_cols, 1.0)
                vaugs.append(v)

            def layernorm_block(dst, src):
                # dst[:, tt, :] = (src[:, tt, :] - m) * rstd
                stats = spool.tile([128, NT, 2], f32, tag="stats")
                for tt in range(NT):
                    st6 = spool.tile([128, 6], f32, tag="st6")
                    nc.vector.bn_stats(st6[:], src[:, tt, :])
                    nc.vector.bn_aggr(stats[:, tt, :], st6[:])
                lnv = spool.tile([128, NT], f32, tag="lnv")
                nc.scalar.activation(lnv[:], stats[:, :, 1], AF.Ln, bias=epsb[:])
                rstd = spool.tile([128, NT], f32, tag="rstd")
                nc.scalar.activation(rstd[:], lnv[:], AF.Exp, scale=-0.5)
                for tt in range(NT):
                    nc.vector.tensor_scalar(
                        dst[:, tt, :], src[:, tt, :],
                        stats[:, tt, 0:1], rstd[:, tt:tt + 1],
                        op0=OP.subtract, op1=OP.mult,
                    )

            def transpose_cast(tb, t8, srcb):
                # srcb [128, NT, C] bf16 -> tb [128, KC, 2N] bf16 (dma xbar)
                # -> t8 [128, KC, 2N] fp8 (DVE cast per token chunk)
                for tt in range(NT):
                    nc.sync.dma_start_transpose(
                        tb[:, :, 128 * tt:128 * tt + 128], srcb[:, tt, :]
                    )
                for half in range(2):
                    nc.vector.tensor_copy(
                        t8[:, :, 256 * half:256 * half + 256],
                        tb[:, :, 256 * half:256 * half + 256],
                    )

            # ---- software-pipelined emission over pairs ----
            # iteration i emits: beta_attn(i), alpha_ln(i+1), beta_proj(i),
            # alpha_qkv(i+1), beta_ln2(i), gamma(i) so pair i+1's LN/transpose
            # chain and pair i's Act-paced phases overlap in each engine's
            # in-order stream. Act-table regions stay [ln_exp ... | gelu].
            S = [dict() for _ in range(NPAIR)]

            def load(p):
                s = S[p]
                s["xt"] = xpool.tile([128, NT, C], f32, name=f"xt{p}", tag="xt")
                for j in range(2):
                    nc.gpsimd.dma_start(
                        s["xt"][:, 2 * j:2 * j + 2, :],
                        xin[2 * p + j].rearrange("(t p) c -> p t c", p=128),
                    )
                s["nzt"] = nzpool.tile([2, 2 * N], bf16, name=f"nzt{p}", tag="nzt")
                nc.gpsimd.dma_start(s["nzt"][:], d_nzT[p])

            def alpha_ln(p):
                s = S[p]
                hb = hpool.tile([128, NT, C], bf16, tag="hb")
                layernorm_block(hb[:], s["xt"])
                s["hTb"] = htpool.tile([128, KC, 2 * N], bf16, name=f"hTb{p}", tag="hTb")
                for tt in range(NT):
                    nc.sync.dma_start_transpose(
                        s["hTb"][:, :, 128 * tt:128 * tt + 128], hb[:, tt, :]
                    )

            def alpha_qkv(p):
                s = S[p]
                nzt = s["nzt"]
                vaug = vaugs[p % 2]
                s["vaug"] = vaug
                hT8 = htpool.tile([128, KC, 2 * N], fp8, tag="hT8")
                for half in range(2):
                    nc.vector.tensor_copy(
                        hT8[:, :, 256 * half:256 * half + 256],
                        s["hTb"][:, :, 256 * half:256 * half + 256],
                    )
                for mt in range(NT):
                    ps = pmm.tile([128, 512], f32, tag="mm")
                    nc.tensor.matmul(
                        ps[:], nzt[:, 128 * mt:128 * mt + 128],
                        seedq[:, 2 * C:3 * C], start=True, stop=False,
                    )
                    for g in range(2):
                        nc.tensor.matmul(
                            ps[:],
                            hT8[:, 2 * g:2 * g + 2, 128 * mt:128 * mt + 128],
                            wqkv8[:, 2 * g:2 * g + 2, 2 * C:3 * C],
                            start=False, stop=(g == 1), perf_mode=PM.DoubleRow,
                        )
                    nc.vector.tensor_scalar(
                        vaug[:, mt, :].rearrange("p (h x) -> p h x", x=66)[:, :, 0:64],
                        ps[:].rearrange("p (h x) -> p h x", x=64),
                        1.0 / WS, None, op0=OP.mult,
                    )
                qkvTs = []
                for hg in range(2):
                    qkvT = qkpool.tile([128, 4, 2 * N], bf16, tag="qkvT")
                    for i, et in enumerate([2 * hg, 2 * hg + 1, 4 + 2 * hg, 5 + 2 * hg]):
                        ps = pmm.tile([128, 512], f32, tag="mm")
                        nc.tensor.matmul(
                            ps[:], seedq[:, 128 * et:128 * et + 128], nzt[:],
                            start=True, stop=False,
                        )
                        for g in range(2):
                            nc.tensor.matmul(
                                ps[:],
                                wqkv8[:, 2 * g:2 * g + 2, 128 * et:128 * et + 128],
                                hT8[:, 2 * g:2 * g + 2, :],
                                start=False, stop=(g == 1), perf_mode=PM.DoubleRow,
                            )
                        if i < 2:  # q: fold attention scale; Act evict
                            nc.scalar.activation(
                                qkvT[:, i, :], ps[:], AF.Copy, scale=SCALE / WS
                            )
                        else:      # k: DVE evict
                            nc.vector.tensor_scalar(
                                qkvT[:, i, :], ps[:], 1.0 / WS, None, op0=OP.mult
                            )
                    qkvTs.append(qkvT)
                s["qkvTs"] = qkvTs

            def beta_attn(p):
                s = S[p]
                vaug = s["vaug"]
                ob = opool.tile([128, NT, C], bf16, name=f"ob{p}", tag="ob")
                s["ob"] = ob
                po2t = pp2.tile([128, 4, H], f32, tag="pv2")
                for bb in range(2):
                    pts = []
                    for hh in range(H):
                        hg, j = hh // 4, hh % 4
                        qkvT = s["qkvTs"][hg]
                        qi, ki, poff = j // 2, 2 + j // 2, 64 * (j % 2)
                        ps_s = pss.tile([128, 2, N], f32, tag="s")
                        for mi in range(2):
                            nc.tensor.matmul(
                                ps_s[:, mi, :], identb[:], biasT[:, mi, hh, :],
                                start=True, stop=False,
                            )
                            mt = 2 * bb + mi
                            nc.tensor.matmul(
                                ps_s[:, mi, :],
                                qkvT[poff:poff + 64, ki, 128 * mt:128 * mt + 128],
                                qkvT[poff:poff + 64, qi, N * bb:N * bb + N],
                                start=False, stop=True,
                            )
                        pt = ptpool.tile([128, 2, N], fp8, tag="pt")
                        nc.scalar.activation(pt[:], ps_s[:], AF.Exp)
                        pts.append(pt)
                    for nt in range(2):
                        po = ppv.tile([128, H, 64], f32, tag="pv")
                        for hh in range(H):
                            lhsT = pts[hh][:, :, 128 * nt:128 * nt + 128]
                            nc.tensor.matmul(
                                po[:, hh, :], lhsT,
                                vaug[:, 2 * bb:2 * bb + 2, 66 * hh:66 * hh + 64],
                                start=True, stop=True, perf_mode=PM.DoubleRow,
                            )
                            nc.tensor.matmul(
                                po2t[:, 2 * bb + nt, hh:hh + 1], lhsT,
                                vaug[:, 2 * bb:2 * bb + 2, 64:65],
                                start=True, stop=True, perf_mode=PM.DoubleRow,
                            )
                        inv = spool.tile([128, H], f32, tag="inv")
                        nc.vector.reciprocal(inv[:], po2t[:, 2 * bb + nt, :])
                        tt = 2 * bb + nt
                        for hh in range(H):
                            if hh % 2 == 0:
                                nc.vector.tensor_scalar(
                                    ob[:, tt, 64 * hh:64 * hh + 64],
                                    po[:, hh, :], inv[:, hh:hh + 1], None,
                                    op0=OP.mult,
                                )
                            else:
                                nc.scalar.activation(
                                    ob[:, tt, 64 * hh:64 * hh + 64],
                                    po[:, hh, :], AF.Copy,
                                    scale=inv[:, hh:hh + 1],
                                )

            def beta_proj_tr(p):
                s = S[p]
                s["oTb"] = htpool.tile([128, KC, 2 * N], bf16, name=f"oTb{p}", tag="oTb")
                for tt in range(NT):
                    nc.sync.dma_start_transpose(
                        s["oTb"][:, :, 128 * tt:128 * tt + 128], s["ob"][:, tt, :]
                    )

            def beta_proj_mm(p):
                s = S[p]
                oT8 = htpool.tile([128, KC, 2 * N], fp8, tag="oT8")
                for half in range(2):
                    nc.vector.tensor_copy(
                        oT8[:, :, 256 * half:256 * half + 256],
                        s["oTb"][:, :, 256 * half:256 * half + 256],
                    )
                for tt in range(NT):
                    ps = pmm.tile([128, 512], f32, tag="mm")
                    nc.tensor.matmul(
                        ps[:], onescol[:], bprojW[:], start=True, stop=False
                    )
                    for g in range(2):
                        nc.tensor.matmul(
                            ps[:],
                            oT8[:, 2 * g:2 * g + 2, 128 * tt:128 * tt + 128],
                            wproj8[:, 2 * g:2 * g + 2, :],
                            start=False, stop=(g == 1), perf_mode=PM.DoubleRow,
                        )
                    nc.vector.scalar_tensor_tensor(
                        s["xt"][:, tt, :], ps[:], 1.0 / WS, s["xt"][:, tt, :],
                        op0=OP.mult, op1=OP.add,
                    )

            def beta_ln2(p):
                s = S[p]
                h2b = hpool.tile([128, NT, C], bf16, tag="hb")
                layernorm_block(h2b[:], s["xt"])
                h2Tb = htpool.tile([128, KC, 2 * N], bf16, tag="h2Tb")
                h2T8 = htpool.tile([128, KC, 2 * N], fp8, tag="h2T8")
                transpose_cast(h2Tb, h2T8, h2b)
                s["h2T8"] = h2T8

            def gamma(p):
                s = S[p]
                h2T8 = s["h2T8"]
                gt8 = gpool.tile([128, KH, 2 * N], fp8, tag="gt8")
                for t_ in range(KH):
                    ps = pmm.tile([128, 512], f32, tag="mm")
                    for g in range(2):
                        nc.tensor.matmul(
                            ps[:],
                            w18[:, 2 * g:2 * g + 2, 128 * t_:128 * t_ + 128],
                            h2T8[:, 2 * g:2 * g + 2, :],
                            start=(g == 0), stop=(g == 1), perf_mode=PM.DoubleRow,
                        )
                    nc.scalar.activation(
                        gt8[:, t_, :], ps[:], AF.Gelu,
                        bias=b1mt[:, t_:t_ + 1], scale=1.0 / WS,
                    )
                for tt in range(NT):
                    psy = pmm.tile([128, 512], f32, tag="mm")
                    for r in range(KH // 2):
                        nc.tensor.matmul(
                            psy[:],
                            gt8[:, 2 * r:2 * r + 2, 128 * tt:128 * tt + 128],
                            w28[:, 2 * r:2 * r + 2, :],
                            start=(r == 0), stop=(r == KH // 2 - 1),
                            perf_mode=PM.DoubleRow,
                        )
                    y = ypool.tile([128, C], f32, tag="y")
                    nc.vector.scalar_tensor_tensor(
                        y[:], psy[:], 1.0 / WS, b2mb[:], op0=OP.mult, op1=OP.add
                    )
                    nc.gpsimd.tensor_add(y[:], y[:], s["xt"][:, tt, :])
                    bi, nt = 2 * p + tt // 2, tt % 2
                    nc.sync.dma_start(yout[bi, 128 * nt:128 * nt + 128, :], y[:])

            load(0)
            load(1)
            alpha_ln(0)
            alpha_qkv(0)
            beta_attn(0)
            for i in range(NPAIR):
                if i + 2 < NPAIR:
                    load(i + 2)
                beta_proj_tr(i)
                if i + 1 < NPAIR:
                    alpha_ln(i + 1)
                beta_proj_mm(i)
                if i + 1 < NPAIR:
                    alpha_qkv(i + 1)
                beta_ln2(i)
                if i + 1 < NPAIR:
                    beta_attn(i + 1)
                gamma(i)

    nc.compile()
    return nc


def _host_prep(x, noise, ns, g1, b1, w_qkv, w_proj, b_proj, rp_table, g2, b2,
               w1, b1m, w2, b2m, rel_index):
    import ml_dtypes
    f = np.float32
    e4 = ml_dtypes.float8_e4m3
    bf = ml_dtypes.bfloat16

    w_qkv = np.asarray(w_qkv, f)
    w_proj = np.asarray(w_proj, f)
    w1 = np.asarray(w1, f)
    w2 = np.asarray(w2, f)
    g1 = np.asarray(g1, f)
    g2 = np.asarray(g2, f)
    b1 = np.asarray(b1, f)
    b2 = np.asarray(b2, f)

    # rel-pos bias gather -> [p, mi, h, n] with m = mi*128+p, bias[n, m, h]
    bias = np.asarray(rp_table, f)[np.asarray(rel_index).reshape(-1)]
    bias = bias.reshape(N, N, H)
    biasT = np.ascontiguousarray(
        bias.transpose(1, 0, 2).reshape(2, 128, N, H).transpose(1, 0, 3, 2)
    ).astype(bf)

    def tiled_T(wf, kt):
        # wf [out, cin] -> [128, kt, out]: [p, k, :] = wf[:, 128k+p]
        wt = np.ascontiguousarray(wf.T)
        return np.ascontiguousarray(
            wt.reshape(kt, 128, wt.shape[1]).transpose(1, 0, 2)
        )

    wq_f = w_qkv * g1[None, :]          # fold g1
    w1_f = w1 * g2[None, :]             # fold g2

    wqkv8 = (WS * tiled_T(wq_f, KC)).astype(e4)
    wproj8 = (WS * tiled_T(w_proj, KC)).astype(e4)
    w18 = (WS * tiled_T(w1_f, KC)).astype(e4)
    w28 = (WS * tiled_T(w2, KH)).astype(e4)

    wb = w_qkv @ b1                      # [3C]
    wsum = w_qkv.sum(axis=1)             # [3C]
    seedqkv = np.ascontiguousarray(
        (WS * np.stack([wb, wsum])).astype(bf)
    )

    b1m_eff = np.asarray(b1m, f) + w1 @ b2
    b1mt = np.ascontiguousarray(b1m_eff.reshape(KH, 128).T)
    b2mb = np.ascontiguousarray(np.broadcast_to(
        np.asarray(b2m, f).reshape(1, -1), (128, C)))
    bprojW = (WS * np.asarray(b_proj, f).reshape(1, C)).astype(bf)

    shared = {
        "wqkv8": wqkv8, "wproj8": wproj8, "w18": w18, "w28": w28,
        "seedqkv": seedqkv, "biasT": biasT, "b1mt": b1mt, "b2mb": b2mb,
        "bprojW": bprojW, "identb": np.eye(128, dtype=f).astype(bf),
    }
    x = np.asarray(x, f)
    nz = np.asarray(noise, f).reshape(B, N) * np.float32(ns)
    in_maps = []
    for c in range(NCORES):
        m = dict(shared)
        m["xin"] = np.ascontiguousarray(x[c * BL:(c + 1) * BL])
        nzc = nz[c * BL:(c + 1) * BL].reshape(NPAIR, 2 * N)
        nzT = np.ones((NPAIR, 2, 2 * N), f)
        nzT[:, 1, :] = nzc
        m["nzT"] = nzT.astype(bf)
        in_maps.append(m)
    return in_maps


def kernel(**inputs):
    from concourse.bass_utils import run_bass_kernel_spmd

    if "nc" not in _CACHE:
        _CACHE["nc"] = _build_nc()
    nc = _CACHE["nc"]
    import time as _time

    in_maps = _host_prep(**inputs)
    _t0 = _time.time()
    res = run_bass_kernel_spmd(nc, in_maps, core_ids=list(range(NCORES)))
    _CACHE["last_run_s"] = _time.time() - _t0
    out = np.concatenate([res.results[c]["yout"] for c in range(NCORES)], axis=0)
    return out.astype(np.float32)
